# revision 22
# baseline (speedup 1.0000x reference)
"""Trainium2 Bass kernel: soft-top-k-masked pseudo-diagonal fully connected layer.

Computes, for x [16, 1024, 768], V [2304, 768], alpha [2304]:
    m  = dykstra_capped_simplex(alpha / 0.01, k=231, 50 iters)        # [2304]
    W[o, j] = m[(o - j) % 2304] * V[(o - j) % 2304, j]                # [2304, 768]
    out = x @ W.T                                                     # [16, 1024, 2304]

Key structure (v2):
  * Host-side layout prep (pure data movement, no arithmetic): x is uploaded
    pre-transposed per-core as xT [768, 2048] fp16; V is uploaded as the
    pre-rolled W_raw [768, 2304] fp16 with W_raw[j, o] = V.T[j, (o-j) % 2304].
    This removes all on-device PE transposes and the DRAM ext roundtrip of the
    previous version.
  * Dykstra reduces to a scalar-bounds recursion: with z = 100*alpha fixed,
        c_i  = clip(z, lo_i, hi_i)          (lo_0 = -inf, hi_0 = +inf)
        A_i  = sum(c_i)
        lo_{i+1} = (A_i - k)/n,   hi_{i+1} = lo_{i+1} + 1
        m = clip(z, lo_50, hi_50) - lo_50
    which is algebraically identical to the reference w-recursion (w_i = z + D_i
    with D_i = -lo_i).  Per iteration: ONE fused DVE clip+accumulate
    (tensor_scalar with two per-partition scalar operands reading the bounds),
    ONE tiny PE matmul against an exact all-ones f32r stationary for the
    cross-partition sum, and ONE tiny DVE op computing both bounds with exact
    f32 constants.  The exact-1.0 stationary matters: the iteration map has
    contraction factor ~0.99, so any systematic per-iteration bias in A/n is
    amplified ~40x into m.
  * The mask is applied to the rolled weights via a skewed broadcast of m
    (m_rep DRAM replicate trick, partition stride 2303 == -1 mod 2304), chunked
    by output columns so the main matmul's first o-chunk starts as soon as m
    and its first msk slices land.
  * Main matmul: o-chunk-major sweeps (5 chunks of <=512), token tiles inner,
    contraction over 6 j-bands accumulated in PSUM.  fp16 operands: 1
    cycle/row streaming, weight loads hidden under the previous matmul.
    Output downloaded fp16 and upcast on host.

Sharding: data-parallel over the 16384 tokens -> 2048 tokens per core on 8
cores; V/alpha replicated (per the sharding hint).
"""

import numpy as np

from concourse import bass, bacc, mybir, tile
from concourse import bass_isa, bass_utils
from concourse.ap import AP

F32 = mybir.dt.float32
F32R = mybir.dt.float32r
F16 = mybir.dt.float16

N_CORES = 8
T_FULL = 16 * 1024          # total tokens
T = T_FULL // N_CORES       # tokens per core = 2048
D = 768                     # in features (contraction)
O = 2304                    # out features
P = 2304                    # total perm (mask length)
KTOP = 231                  # top-k target
NUM_ITER = 50
INV_LR = 100.0              # 1 / 0.01
INV_N = 1.0 / float(P)
KK_LO = float(np.float32(KTOP) / np.float32(P))          # k/n
KK_HI = float((np.float32(KTOP) - np.float32(P)) / np.float32(P))  # (k-n)/n

NT = T // 128               # 16 token tiles per core
NJ = D // 128               # 6 contraction bands
NF = P // 128               # 18 = columns of the [128, 18] dykstra layout
# o-chunks for the main matmul (one PSUM bank each)
O_CHUNKS = [(0, 512), (512, 1024), (1024, 1536), (1536, 2048), (2048, 2304)]
BIG = 1.0e30


def build_program():
    nc = bacc.Bacc("TRN2", target_bir_lowering=False, debug=False,
                   num_devices=N_CORES)

    xT_d = nc.dram_tensor("xT", [D, T], F16, kind="ExternalInput")
    wraw_d = nc.dram_tensor("wraw", [D, O], F16, kind="ExternalInput")
    alphaT_d = nc.dram_tensor("alphaT", [128, NF], F32, kind="ExternalInput")
    ident_d = nc.dram_tensor("ident", [128, 128], F16, kind="ExternalInput")
    out_d = nc.dram_tensor("out", [T, O], F16, kind="ExternalOutput")

    mtmp_d = nc.dram_tensor("m_tmp", [P], F16, kind="Internal")
    mrep_d = nc.dram_tensor("m_rep", [130 * P], F16, kind="Internal")

    out_r = out_d.ap().rearrange("(n p) o -> n p o", p=128)  # [16, 128, 2304]

    with tile.TileContext(nc) as tc:
        with (
            tc.tile_pool(name="const", bufs=1) as constp,
            tc.tile_pool(name="small", bufs=1) as small,
            tc.tile_pool(name="xt", bufs=NJ) as xtp,
            tc.tile_pool(name="wt", bufs=NJ) as wtp,
            tc.tile_pool(name="msk", bufs=3 * NJ) as mskp,
            tc.tile_pool(name="ost", bufs=6) as ostp,
            tc.tile_pool(name="mmps", bufs=4, space="PSUM") as mmps,
            tc.tile_pool(name="dk", bufs=2, space="PSUM") as dkp,
        ):
            # ---- alpha first: it alone gates the Dykstra critical path ----
            al_t = small.tile([128, NF], F32, tag="al")
            nc.gpsimd.dma_start(al_t[:], alphaT_d.ap())

            # ---- constants / bulk loads (off the dykstra critical path) ----
            ident = constp.tile([128, 128], F16)
            nc.scalar.dma_start(ident[:], ident_d.ap())
            kk2n = constp.tile([128, 2], F32)
            nc.vector.memset(kk2n[:, 0:1], KK_LO)
            nc.vector.memset(kk2n[:, 1:2], KK_HI)

            xt = [xtp.tile([128, T], F16, tag="xt", name=f"xt{b}")
                  for b in range(NJ)]
            wt = [wtp.tile([128, O], F16, tag="wt", name=f"wt{b}")
                  for b in range(NJ)]
            for b in range(NJ):
                nc.scalar.dma_start(xt[b][:], xT_d.ap()[128 * b:128 * (b + 1), :])
            for b in range(NJ):
                nc.sync.dma_start(wt[b][:], wraw_d.ap()[128 * b:128 * (b + 1), :])

            # ---- Dykstra scalar-bounds recursion ----
            z = small.tile([128, NF], F32, tag="z")
            c = small.tile([128, NF], F32, tag="c")
            red = small.tile([128, 1], F32, tag="red")
            bounds = small.tile([128, 2], F32, tag="bounds")  # [lo, hi]
            m16 = small.tile([128, NF], F16, tag="m16")

            nc.vector.tensor_scalar_mul(z[:], al_t[:], INV_LR)
            nc.vector.memset(bounds[:, 0:1], -BIG)
            nc.vector.memset(bounds[:, 1:2], BIG)
            lo_bcast = bounds[:, 0:1].broadcast_to([128, NF])
            ared = small.tile([128, 1], F32, tag="ared")
            for i in range(NUM_ITER):
                # c = (z min hi) max lo ; red = per-partition row sums
                nc.vector.scalar_tensor_tensor(c[:], z[:], bounds[:, 1:2],
                                               lo_bcast,
                                               op0=mybir.AluOpType.min,
                                               op1=mybir.AluOpType.max,
                                               accum_out=red[:])
                # A = cross-partition sum, broadcast to all partitions
                nc.gpsimd.partition_all_reduce(ared[:], red[:], 128,
                                               bass_isa.ReduceOp.add)
                # bounds = (A * 1/n) - [k/n, (k-n)/n]
                nc.vector.scalar_tensor_tensor(bounds[:],
                                               ared[:].broadcast_to([128, 2]),
                                               INV_N, kk2n[:],
                                               op0=mybir.AluOpType.mult,
                                               op1=mybir.AluOpType.subtract)
            # m = clip(z, lo, hi) - lo   (fresh clip with final bounds)
            nc.vector.scalar_tensor_tensor(c[:], z[:], bounds[:, 1:2],
                                           lo_bcast,
                                           op0=mybir.AluOpType.min,
                                           op1=mybir.AluOpType.max)
            nc.vector.tensor_scalar(m16[:], c[:], bounds[:, 0:1], None,
                                    op0=mybir.AluOpType.subtract)

            # ---- m -> DRAM natural order -> 130x replicate for skewed reads
            # (skew uses positive partition stride P-1 == -1 mod P; the
            # replicate is split across two queues to halve its latency)
            mt_ps = dkp.tile([NF, 128], F16, tag="dk")
            nc.tensor.transpose(mt_ps[:], m16[:], ident[:])
            mt_sb = small.tile([NF, 128], F16, tag="mtsb")
            nc.scalar.copy(mt_sb[:], mt_ps[:])
            mw0 = nc.gpsimd.dma_start(
                mtmp_d.ap().rearrange("(f p) -> f p", p=128), mt_sb[:])
            mr0 = nc.gpsimd.dma_start(
                AP(mrep_d, 0, [[P, 65], [1, P]]),
                AP(mtmp_d, 0, [[0, 65], [1, P]]))
            mr1 = nc.sync.dma_start(
                AP(mrep_d, 65 * P, [[P, 65], [1, P]]),
                AP(mtmp_d, 0, [[0, 65], [1, P]]))
            tile.add_dep_helper(mr0.ins, mw0.ins, reason="m_tmp RAW")
            tile.add_dep_helper(mr1.ins, mw0.ins, reason="m_tmp RAW")

            # ---- skewed m broadcast: msk[b][dj, o] = m[(o - dj - j0) % P],
            # chunk-major across three queues so early o-chunks land first.
            # One tile per (band, slice-group) so a mask multiply only waits
            # for its own slice's DMA (dep tracking is tile-granular).
            SLICES = [(0, 512), (512, 1024), (1024, O)]
            msk = [[mskp.tile([128, s1 - s0], F16, tag="msk",
                              name=f"msk{b}_{g}")
                    for g, (s0, s1) in enumerate(SLICES)]
                   for b in range(NJ)]
            mq = [nc.gpsimd, nc.sync, nc.scalar]
            for g, (s0, s1) in enumerate(SLICES):
                for b in range(NJ):
                    j0 = 128 * b
                    r = mq[b % 3].dma_start(
                        msk[b][g][:],
                        AP(mrep_d, P - j0 + s0, [[P - 1, 128], [1, s1 - s0]]))
                    tile.add_dep_helper(r.ins, mr0.ins, reason="m_rep RAW")
                    tile.add_dep_helper(r.ins, mr1.ins, reason="m_rep RAW")

            # ---- apply mask: wt[b] *= msk[b], chunk-major so chunk 0 is ready
            # first; alternate vector/gpsimd
            for ci, (o0, o1) in enumerate(O_CHUNKS):
                g = min(ci, 2)
                g0 = o0 - SLICES[g][0]
                g1 = o1 - SLICES[g][0]
                for b in range(NJ):
                    eng = nc.vector if (ci * NJ + b) % 2 == 0 else nc.gpsimd
                    eng.tensor_tensor(wt[b][:, o0:o1], wt[b][:, o0:o1],
                                      msk[b][g][:, g0:g1],
                                      op=mybir.AluOpType.mult)

            # ---- main matmul: o-chunk sweeps, token tiles inner ----
            flip = 0
            for ci, (o0, o1) in enumerate(O_CHUNKS):
                cw = o1 - o0
                for tt in range(NT):
                    ps = mmps.tile([128, 512], F32, tag="mm")
                    for b in range(NJ):
                        nc.tensor.matmul(
                            ps[:, 0:cw],
                            xt[b][:, 128 * tt:128 * (tt + 1)],
                            wt[b][:, o0:o1],
                            start=(b == 0), stop=(b == NJ - 1),
                        )
                    ost = ostp.tile([128, 512], F16, tag="ost")
                    if flip % 2 == 0:
                        nc.scalar.copy(ost[:, 0:cw], ps[:, 0:cw])
                        nc.scalar.dma_start(out_r[tt][:, o0:o1], ost[:, 0:cw])
                    else:
                        nc.vector.tensor_copy(ost[:, 0:cw], ps[:, 0:cw])
                        nc.sync.dma_start(out_r[tt][:, o0:o1], ost[:, 0:cw])
                    flip += 1

    nc.compile()
    return nc


_CACHE = {}


def _get_program():
    if "nc" not in _CACHE:
        _CACHE["nc"] = build_program()
    return _CACHE["nc"]


def _host_inputs(x, V, alpha):
    """Pure layout prep (transpose/cast/roll); no arithmetic on values."""
    xf = np.ascontiguousarray(x.reshape(T_FULL, D))
    VT16 = np.ascontiguousarray(V.T.astype(np.float16))          # [768, 2304]
    idx = (np.arange(O)[None, :] - np.arange(D)[:, None]) % P    # [768, 2304]
    wraw = np.ascontiguousarray(np.take_along_axis(VT16, idx, axis=1))
    alphaT = np.ascontiguousarray(
        alpha.astype(np.float32).reshape(NF, 128).T)             # [128, 18]
    ident = np.eye(128, dtype=np.float16)
    maps = []
    for cid in range(N_CORES):
        xT = np.ascontiguousarray(
            xf[T * cid:T * (cid + 1)].T.astype(np.float16))      # [768, 2048]
        maps.append({"xT": xT, "wraw": wraw, "alphaT": alphaT,
                     "ident": ident})
    return maps


def kernel(x, V, alpha):
    nc = _get_program()
    in_maps = _host_inputs(x, V, alpha)
    res = bass_utils.run_bass_kernel_spmd(nc, in_maps,
                                          core_ids=list(range(N_CORES)))
    out = np.concatenate(
        [res.results[c]["out"].astype(np.float32) for c in range(N_CORES)],
        axis=0)
    return out.reshape(16, 1024, O)
